# revision 1
# baseline (speedup 1.0000x reference)
"""Trainium2 Bass kernel for a 1-D correlation volume (stereo cost volume).

Problem: out[n, i, h, w] = (1/C) * sum_c x1[n,c,h,w] * x2[n,c,h,w-i],
zero where w-i < 0, for i in 0..D (D=64).
Shapes: x1, x2 = [8, 128, 128, 256] f32; out = [8, 65, 128, 256] f32.

Sharding: data-parallel over the batch dim — each of the 8 NeuronCores
processes one sample end to end (no collectives).

Per-core algorithm (v2)
-----------------------
The contraction over c maps onto the TensorEngine as a banded Gram
matmul. For w-tile ts (128 wide), per row h:
    band[p, w'] = sum_c x1[c, ts+p] * x2[c, w']
computed as ONE matmul with the full x2 row as the moving operand
(N=256) in float32r mode — 1 cycle/row instead of 4 for plain fp32,
with no input cast. Band coords col = w' - ts + 64, so output
out[i, h, ts+p] = band[p, col=p+64-i] / C: the 65 needed values per p
sit on diagonals col - p in [0, 64].

Extracting those diagonals needs per-partition varying offsets, which
no SBUF engine can address, so the band goes through a DRAM scratch
where flat addressing collapses a diagonal into a plain strided read.
v2 cuts the round-trip traffic vs v1:
  * scratch is bf16 (band values ~N(0,128); 2e-2 tolerance is easy),
  * only the used trapezoid is written: partition blocks of 32 rows
    only store their 96-column window (instead of the full 192-wide
    band), halving write bytes again,
  * 1/C (= 2^-7, exact) is folded into the PSUM->SBUF drain,
  * drains are split across the Activation and Pool engines,
  * the h dimension is processed in two halves with separate
    scratches, so the extraction/repack/store of half 0 overlaps the
    band compute of half 1.
Scratch layout per (half, tile): [4 pblk, 32 p', 64 hh, 96 col'] bf16;
fiber read addr = hh*96 + pblk*(32*64*96) + p'*(64*96+1) + j gives
ft[hh, pblk, p', j] = band value for i = 64-j at w = ts + 32*pblk + p'.
A DVE repack turns that into gt[hh, j, w-in-tile] and one DMA per
(half, tile) stores to out with a negative i-stride (i = 64-j).
"""

import numpy as np

import concourse.bass as bass
import concourse.tile as tile
from concourse import bacc, mybir
from concourse.bass_utils import run_bass_kernel_spmd

# Problem constants (hardcoded per the harness contract).
B = 8          # batch == number of cores
C = 128        # channels (matmul K)
H = 128        # rows
W = 256        # cols
D = 64         # max disparity
ND = D + 1     # number of disparities (65)
T = 128        # w-tile size (output partition dim of the band matmul)
NT = W // T    # 2 w-tiles
BANDC = T + D  # 192 band columns per tile
HB = 8         # h rows per load/staging block
HH = 64        # rows per h-half (2 halves, pipelined)
PB = 32        # partition-block size for trapezoid strip writes
NPB = T // PB  # 4 partition blocks
SW = PB + D    # 96: strip width (col window per partition block)

F32 = mybir.dt.float32
F32R = mybir.dt.float32r
BF16 = mybir.dt.bfloat16


def _corr_body(tc, out_d, x1_d, x2_d):
    nc = tc.nc
    with (
        tc.tile_pool(name="io", bufs=2) as io_pool,
        tc.tile_pool(name="band", bufs=2) as band_pool,
        tc.tile_pool(name="psum", bufs=8, space="PSUM") as psum_pool,
        tc.tile_pool(name="fib", bufs=2) as fib_pool,
        tc.tile_pool(name="out", bufs=2) as out_pool,
        tc.tile_pool(name="dram", bufs=1, space="DRAM") as dram_pool,
    ):
        # DRAM scratch per (half, tile): [NPB, HH, PB, SW] bf16 — h-major
        # within each partition block, so the sheared fiber readback walks
        # nearly-sequential addresses (fiber p' stride is SW+1 elements).
        scr = [
            [
                dram_pool.tile(
                    [NPB, HH, PB, SW], BF16, tag=f"scr{hf}{t}", name=f"scr{hf}{t}"
                )
                for t in range(NT)
            ]
            for hf in range(2)
        ]

        for hf in range(2):
            for hb in range(hf * HH, hf * HH + HH, HB):
                x1t = io_pool.tile([C, HB, W], F32R, tag="x1t")
                nc.sync.dma_start(x1t[:], x1_d[:, hb : hb + HB, :].bitcast(F32R))
                x2t = io_pool.tile([C, HB, W], F32R, tag="x2t")
                nc.sync.dma_start(x2t[:], x2_d[:, hb : hb + HB, :].bitcast(F32R))

                bb = [
                    band_pool.tile([T, HB, BANDC], BF16, tag=f"bb{t}", name=f"bb{t}")
                    for t in range(NT)
                ]
                # Tile 0 band cols 0:64 are w' < 0 -> zero padding.
                nc.gpsimd.memset(bb[0][:, :, 0:D], 0.0)

                for hl in range(HB):
                    # One matmul per w-tile: x1 tile is the stationary
                    # operand, the full x2 row streams (N=256 -> fp32r
                    # runs at 1 cycle/row).
                    rhs = x2t[:, hl, :]
                    pt0 = psum_pool.tile([T, W], F32, tag="pt")
                    nc.tensor.matmul(
                        pt0[:],
                        x1t[:, hl, 0:T],
                        rhs,
                        start=True,
                        stop=True,
                    )
                    # band col = w' + 64 for tile 0: drain w' in [0,128).
                    nc.scalar.mul(bb[0][:, hl, D:BANDC], pt0[:, 0:T], 1.0 / C)

                    pt1 = psum_pool.tile([T, W], F32, tag="pt")
                    nc.tensor.matmul(
                        pt1[:],
                        x1t[:, hl, T:W],
                        rhs,
                        start=True,
                        stop=True,
                    )
                    # band col = w' - 64 for tile 1: drain w' in [64,256).
                    nc.scalar.mul(bb[1][:, hl, :], pt1[:, D:W], 1.0 / C)

                # Trapezoid strip writes: partition block k only stores
                # band cols [32k, 32k+96).
                for t in range(NT):
                    for k in range(NPB):
                        dst = bass.AP(
                            scr[hf][t].tensor,
                            scr[hf][t].offset
                            + k * (HH * PB * SW)
                            + (hb - hf * HH) * (PB * SW),
                            [[SW, PB], [PB * SW, HB], [1, SW]],
                        )
                        nc.sync.dma_start(
                            dst, bb[t][k * PB : (k + 1) * PB, :, k * PB : k * PB + SW]
                        )

            # Extraction for this h-half. The strips are read back
            # CONTIGUOUSLY (6 KB descriptors, line rate) with h in the
            # partition dim; the diagonal shear then happens in the DVE
            # repack as a per-partition-uniform strided access pattern:
            # element (j, p') sits at free offset k*PB*SW + p'*(SW+1) + j.
            # gt[hh, j, w] with i = 64 - j; one full-width store per half.
            gt = out_pool.tile([HH, ND, W], F32, tag="gt", bufs=1)
            for t in range(NT):
                ft = fib_pool.tile([HH, NPB, PB, SW], BF16, tag="ft")
                src = bass.AP(
                    scr[hf][t].tensor,
                    scr[hf][t].offset,
                    [[PB * SW, HH], [HH * PB * SW, NPB], [1, PB * SW]],
                )
                nc.sync.dma_start(ft[:], src)

                for k in range(NPB):
                    shear = bass.AP(
                        ft.tensor,
                        ft.offset + k * (PB * SW),
                        [[NPB * PB * SW, HH], [1, ND], [SW + 1, PB]],
                    )
                    nc.vector.tensor_copy(
                        gt[:, :, t * T + k * PB : t * T + (k + 1) * PB],
                        shear,
                    )

            dst = bass.AP(
                out_d,
                D * H * W + hf * HH * W,
                [[W, HH], [-H * W, ND], [1, W]],
            )
            nc.sync.dma_start(dst, gt[:])


_NC_CACHE = None


def _build_nc():
    global _NC_CACHE
    if _NC_CACHE is not None:
        return _NC_CACHE
    nc = bacc.Bacc("TRN2")
    x1_d = nc.declare_dram_parameter("x1", [C, H, W], F32, isOutput=False)
    x2_d = nc.declare_dram_parameter("x2", [C, H, W], F32, isOutput=False)
    out_d = nc.declare_dram_parameter("out", [ND, H, W], F32, isOutput=True)
    with tile.TileContext(nc) as tc:
        _corr_body(tc, out_d, x1_d, x2_d)
    nc.finalize()
    _NC_CACHE = nc
    return nc


def kernel(x1: np.ndarray, x2: np.ndarray) -> np.ndarray:
    assert x1.shape == (B, C, H, W) and x2.shape == (B, C, H, W)
    nc = _build_nc()
    in_maps = [
        {
            "x1": np.ascontiguousarray(x1[n], dtype=np.float32),
            "x2": np.ascontiguousarray(x2[n], dtype=np.float32),
        }
        for n in range(B)
    ]
    res = run_bass_kernel_spmd(nc, in_maps, list(range(B)))
    return np.stack([res.results[n]["out"] for n in range(B)], axis=0)



# revision 7
# speedup vs baseline: 1.1566x; 1.1566x over previous
"""Trainium2 Bass kernel for a 1-D correlation volume (stereo cost volume).

Problem: out[n, i, h, w] = (1/C) * sum_c x1[n,c,h,w] * x2[n,c,h,w-i],
zero where w-i < 0, for i in 0..D (D=64).
Shapes: x1, x2 = [8, 128, 128, 256] f32; out = [8, 65, 128, 256] f32.

Sharding: data-parallel over the batch dim - each of the 8 NeuronCores
processes one sample end to end (no collectives).

Per-core algorithm (v3)
-----------------------
The contraction over c maps onto the TensorEngine as a banded Gram
matmul. For w-tile ts (128 wide), per row h:
    band[p, w'] = sum_c x1[c, ts+p] * x2[c, w']
computed as ONE matmul with the full x2 row as the moving operand
(N=256) in float32r mode. Band coords col = w' - ts + 64, so output
out[i, h, ts+p] = band[p, col=p+64-i] / C: the 65 needed values per p
sit on diagonals col - p in [0, 64]. Extracting those diagonals needs
per-partition varying offsets, which no SBUF engine can address, so
the band goes through a DRAM scratch where flat addressing collapses a
diagonal into a plain strided read.

v3 vs v2 (profile-driven):
  * Scratch layout is h-MAJOR: scr[h, k, p', c]. The readback then
    reads 24 KB fully-contiguous per partition (line-rate ~360 GB/s,
    vs 200 GB/s for v2's 6 KB strided runs). The strided (192 B run)
    side of the transpose is moved to the strip WRITES, which the
    profile showed run at ~214 GB/s aggregate either way.
  * The h dimension is processed in 4 chunks of 32 rows; chunk c's
    extraction (readback -> DVE shear -> store) is emitted interleaved
    into chunk c+1's banding so the DMA rings never drain. v2's
    2-half pipeline exposed an 11 us dead zone at the tail.
  * DMA issue is split across sequencers to kill issue convoys (v2
    issued everything on Sync in-order; a waiting DMA blocked all
    later independent DMAs - measured 10 us median issue wait):
    loads/readbacks/stores on Sync, strip writes on ACT right after
    the drains that produce their data (engine-local dep, zero wait).
  * PSUM is drained in 4-row groups ([128, 4, 256] tiles, 2 banks) -
    halves ACT instruction count+overhead vs per-row drains.
  * The DVE shear reads j reversed so gt holds [h, i, w] directly and
    the output store walks ascending DRAM addresses.
Scratch per (chunk, t): [CH=32 h][NPB=4 k][PB=32 p'][SW=96 c] bf16.
Strip (hb, t, k) stores band cols [32k, 32k+96) of partitions
[32k, 32k+32); readback ft[h, k, p', c] is one contiguous block per
partition; shear gt[h, i, ts+32k+p'] = ft[h, k, p', p' + 64 - i] is a
per-partition-uniform strided DVE copy (offset p'*97 + 64 - i).
"""

import numpy as np

import concourse.bass as bass
import concourse.tile as tile
from concourse import bacc, mybir
from concourse.bass_utils import run_bass_kernel_spmd

# Problem constants (hardcoded per the harness contract).
B = 8          # batch == number of cores
C = 128        # channels (matmul K)
H = 128        # rows
W = 256        # cols
D = 64         # max disparity
ND = D + 1     # number of disparities (65)
T = 128        # w-tile size (output partition dim of the band matmul)
NT = W // T    # 2 w-tiles
BANDC = T + D  # 192 band columns per tile
HB = 16        # h rows per load/banding block
CH = 32        # h rows per extraction chunk (4 chunks, pipelined)
NCH = H // CH
PB = 32        # partition-block size for trapezoid strip writes
NPB = T // PB  # 4 partition blocks
SW = PB + D    # 96: strip width (col window per partition block)
RG = 4         # rows per PSUM drain group

F32 = mybir.dt.float32
F32R = mybir.dt.float32r
BF16 = mybir.dt.bfloat16

# scratch strides (elements) for layout [CH h][NPB k][PB p'][SW c]
S_H = NPB * PB * SW   # 12288
S_K = PB * SW         # 3072
S_P = SW              # 96


def _corr_body(tc, out_d, x1_d, x2_d):
    nc = tc.nc
    with (
        tc.tile_pool(name="io", bufs=2) as io_pool,
        tc.tile_pool(name="band", bufs=2) as band_pool,
        tc.tile_pool(name="psum", bufs=4, space="PSUM") as psum_pool,
        tc.tile_pool(name="fib", bufs=1) as fib_pool,
        tc.tile_pool(name="out", bufs=1) as out_pool,
        tc.tile_pool(name="dram", bufs=1, space="DRAM") as dram_pool,
    ):
        scr = [
            [
                dram_pool.tile(
                    [CH, NPB, PB, SW], BF16, tag=f"scr{c}{t}", name=f"scr{c}{t}"
                )
                for t in range(NT)
            ]
            for c in range(NCH)
        ]
        gts = {}

        def banding_block(ch, hb):
            x1t = io_pool.tile([C, HB, W], F32R, tag="x1t")
            nc.sync.dma_start(x1t[:], x1_d[:, hb : hb + HB, :].bitcast(F32R))
            x2t = io_pool.tile([C, HB, W], F32R, tag="x2t")
            nc.sync.dma_start(x2t[:], x2_d[:, hb : hb + HB, :].bitcast(F32R))

            bb = [
                band_pool.tile([T, HB, BANDC], BF16, tag=f"bb{t}", name=f"bb{t}")
                for t in range(NT)
            ]
            # Tile 0 band cols 0:64 are w' < 0 -> zero padding.
            nc.gpsimd.memset(bb[0][:, :, 0:D], 0.0)

            for g in range(HB // RG):
                pts = [
                    psum_pool.tile([T, RG, W], F32, tag="pt", name=f"pt{t}")
                    for t in range(NT)
                ]
                for r in range(RG):
                    hl = g * RG + r
                    rhs = x2t[:, hl, :]
                    nc.tensor.matmul(
                        pts[0][:, r, :], x1t[:, hl, 0:T], rhs, start=True, stop=True
                    )
                    nc.tensor.matmul(
                        pts[1][:, r, :], x1t[:, hl, T:W], rhs, start=True, stop=True
                    )
                # Grouped drains with the 1/C (= 2^-7, exact) scale folded in.
                # band col = w' + 64 for tile 0: keep w' in [0,128).
                nc.scalar.mul(
                    bb[0][:, g * RG : (g + 1) * RG, D:BANDC],
                    pts[0][:, :, 0:T],
                    1.0 / C,
                )
                # band col = w' - 64 for tile 1: keep w' in [64,256).
                nc.scalar.mul(
                    bb[1][:, g * RG : (g + 1) * RG, :],
                    pts[1][:, :, D:W],
                    1.0 / C,
                )

            # Trapezoid strip writes, issued from ACT so they queue right
            # behind this block's drains (engine-local dependency). Strip
            # (t, k) = band cols [32k, 32k+96) of partitions [32k, 32k+32),
            # h-major dst: per-partition runs of SW*2 bytes.
            hoff = hb - (ch * CH)
            for t in range(NT):
                for k in range(NPB):
                    dst = bass.AP(
                        scr[ch][t].tensor,
                        scr[ch][t].offset + hoff * S_H + k * S_K,
                        [[S_P, PB], [S_H, HB], [1, SW]],
                    )
                    nc.scalar.dma_start(
                        dst, bb[t][k * PB : (k + 1) * PB, :, k * PB : k * PB + SW]
                    )

        def readback(ch, t):
            # One fully-contiguous 768 KB read: 24 KB per partition.
            ft = fib_pool.tile([CH, NPB, PB, SW], BF16, tag="ft", name="ft")
            src = bass.AP(
                scr[ch][t].tensor,
                scr[ch][t].offset,
                [[S_H, CH], [S_K, NPB], [S_P, PB], [1, SW]],
            )
            nc.sync.dma_start(ft[:], src)
            return ft

        def repack(ch, t, ft):
            # DVE shear: gt[h, i, ts+32k+p'] = ft[h, k, p', p' + 64 - i],
            # free offset k*S_K + p'*(SW+1) + (64 - i): per-partition-uniform.
            if t == 0:
                gts[ch] = out_pool.tile([CH, ND, W], F32, tag="gt", name="gt")
            gt = gts[ch]
            for k in range(NPB):
                shear = bass.AP(
                    ft.tensor,
                    ft.offset + k * S_K + D,
                    [[S_H, CH], [-1, ND], [SW + 1, PB]],
                )
                nc.vector.tensor_copy(
                    gt[:, :, t * T + k * PB : t * T + (k + 1) * PB],
                    shear,
                )

        def store(ch):
            # Ascending-address store: gt already holds [h, i, w].
            dst = bass.AP(
                out_d,
                ch * CH * W,
                [[W, CH], [H * W, ND], [1, W]],
            )
            nc.sync.dma_start(dst, gts[ch][:])

        # Software-pipelined emission: chunk ch's banding interleaved with
        # chunk ch-1's extraction so the DMA rings never drain.
        fts = {}
        for ch in range(NCH):
            for bi, hb in enumerate(range(ch * CH, ch * CH + CH, HB)):
                banding_block(ch, hb)
                if ch > 0:
                    if bi == 0:
                        ft0 = readback(ch - 1, 0)
                        repack(ch - 1, 0, ft0)
                        fts[ch - 1] = readback(ch - 1, 1)
                    else:
                        repack(ch - 1, 1, fts.pop(ch - 1))
                        store(ch - 1)
        ft0 = readback(NCH - 1, 0)
        repack(NCH - 1, 0, ft0)
        ft1 = readback(NCH - 1, 1)
        repack(NCH - 1, 1, ft1)
        store(NCH - 1)


_NC_CACHE = None


def _build_nc():
    global _NC_CACHE
    if _NC_CACHE is not None:
        return _NC_CACHE
    nc = bacc.Bacc("TRN2")
    x1_d = nc.declare_dram_parameter("x1", [C, H, W], F32, isOutput=False)
    x2_d = nc.declare_dram_parameter("x2", [C, H, W], F32, isOutput=False)
    out_d = nc.declare_dram_parameter("out", [ND, H, W], F32, isOutput=True)
    with tile.TileContext(nc) as tc:
        _corr_body(tc, out_d, x1_d, x2_d)
    nc.finalize()
    _NC_CACHE = nc
    return nc


def kernel(x1: np.ndarray, x2: np.ndarray) -> np.ndarray:
    assert x1.shape == (B, C, H, W) and x2.shape == (B, C, H, W)
    nc = _build_nc()
    in_maps = [
        {
            "x1": np.ascontiguousarray(x1[n], dtype=np.float32),
            "x2": np.ascontiguousarray(x2[n], dtype=np.float32),
        }
        for n in range(B)
    ]
    res = run_bass_kernel_spmd(nc, in_maps, list(range(B)))
    return np.stack([res.results[n]["out"] for n in range(B)], axis=0)
